# revision 4
# baseline (speedup 1.0000x reference)
"""Trainium2 Bass kernel for Llama4TextExperts-style grouped MoE FFN (SwiGLU).

Full-input contract: kernel(**inputs) takes the complete unsharded tensors and
returns the full [4096, 1024] output. Internally: expert-parallel across the 8
NeuronCores — core e gets expert e's three weight matrices and that expert's
512-token group (tokens arrive pre-sorted by expert with equal group sizes).
All routing / transposition is done host-side in numpy; no collectives needed.

Per-core device program (three GEMMs, ~6.4 GFLOP):
  phase 1: gate^T = Wg^T-stationary @ x^T, up^T likewise; SwiGLU fused on
           ACT (Silu) + DVE (mul) into h^T [I, T] bf16 resident in SBUF.
  phase 2: y = h @ Wd with h^T slices stationary, Wd streaming from its
           natural [I, H] DRAM layout; y lands untransposed in PSUM.
"""

import numpy as np
import ml_dtypes

import concourse.bass as bass
import concourse.mybir as mybir
import concourse.tile as tile
from concourse import bacc
from concourse.bass_utils import run_bass_kernel_spmd

# Problem shape (hardcoded per contract)
E = 8          # experts == cores
T = 512        # tokens per expert group
H = 1024       # hidden
I = 2048       # intermediate
P = 128        # partitions
KT = H // P    # 8  k-tiles over hidden
IT = I // P    # 16 i-tiles over intermediate
WB = 4         # i-blocks of 512 columns for gate/up weight streaming
MT = T // P    # 4  token tiles

BF16 = mybir.dt.bfloat16
F32 = mybir.dt.float32

_compiled = None  # (nc, ) cached across calls


def _build():
    nc = bacc.Bacc(None)
    xT_d = nc.declare_dram_parameter("xT", [H, T], BF16, isOutput=False)
    wg_d = nc.declare_dram_parameter("wg", [H, I], BF16, isOutput=False)
    wu_d = nc.declare_dram_parameter("wu", [H, I], BF16, isOutput=False)
    wd_d = nc.declare_dram_parameter("wd", [I, H], BF16, isOutput=False)
    y_d = nc.declare_dram_parameter("y", [T, H], F32, isOutput=True)

    xT_r = xT_d.rearrange("(ko p) t -> p ko t", p=P)     # [128, 8, 512]
    wg_r = wg_d.rearrange("(ko p) i -> p ko i", p=P)     # [128, 8, 2048]
    wu_r = wu_d.rearrange("(ko p) i -> p ko i", p=P)
    wd_r = wd_d.rearrange("(io p) h -> p io h", p=P)     # [128, 16, 1024]

    with tile.TileContext(nc) as tc:
        with (
            tc.tile_pool(name="xpool", bufs=1) as xpool,
            tc.tile_pool(name="wdpool", bufs=1) as wdpool,
            tc.tile_pool(name="hpool", bufs=1) as hpool,
            tc.tile_pool(name="wpool", bufs=2) as wpool,
            tc.tile_pool(name="spool", bufs=3) as spool,
            tc.tile_pool(name="ypool", bufs=2) as ypool,
            tc.tile_pool(name="psum", bufs=2, space="PSUM") as psum,
        ):
            xT_sb = xpool.tile([P, KT, T], BF16)
            nc.sync.dma_start(xT_sb[:], xT_r[:])

            hT_sb = hpool.tile([P, IT, T], BF16)
            wd_sb = wdpool.tile([P, IT, H], BF16)

            for wb in range(WB):
                wg_blk = wpool.tile([P, KT, 512], BF16, tag="wg")
                nc.sync.dma_start(wg_blk[:], wg_r[:, :, wb * 512:(wb + 1) * 512])
                wu_blk = wpool.tile([P, KT, 512], BF16, tag="wu")
                nc.sync.dma_start(wu_blk[:], wu_r[:, :, wb * 512:(wb + 1) * 512])
                # stream a quarter of Wd alongside each gate/up block so the
                # phase-2 weights arrive without front-loading the DMA queue
                nc.sync.dma_start(
                    wd_sb[:, wb * 4:(wb + 1) * 4, :],
                    wd_r[:, wb * 4:(wb + 1) * 4, :],
                )

                for itl in range(4):
                    it = wb * 4 + itl
                    cs = slice(itl * P, (itl + 1) * P)
                    pg = psum.tile([P, T], F32, tag="pg")
                    pu = psum.tile([P, T], F32, tag="pu")
                    for kt in range(KT):
                        nc.tensor.matmul(
                            pg[:], wg_blk[:, kt, cs], xT_sb[:, kt, :],
                            start=(kt == 0), stop=(kt == KT - 1),
                        )
                    for kt in range(KT):
                        nc.tensor.matmul(
                            pu[:], wu_blk[:, kt, cs], xT_sb[:, kt, :],
                            start=(kt == 0), stop=(kt == KT - 1),
                        )
                    sg = spool.tile([P, T], F32)
                    nc.scalar.activation(
                        sg[:], pg[:], mybir.ActivationFunctionType.Silu
                    )
                    nc.vector.tensor_mul(hT_sb[:, it, :], sg[:], pu[:])

            for mt in range(MT):
                py0 = psum.tile([P, 512], F32, tag="py0")
                py1 = psum.tile([P, 512], F32, tag="py1")
                ms = slice(mt * P, (mt + 1) * P)
                for it in range(IT):
                    lhsT = hT_sb[:, it, ms]
                    nc.tensor.matmul(
                        py0[:], lhsT, wd_sb[:, it, 0:512],
                        start=(it == 0), stop=(it == IT - 1),
                    )
                    nc.tensor.matmul(
                        py1[:], lhsT, wd_sb[:, it, 512:1024],
                        start=(it == 0), stop=(it == IT - 1),
                    )
                y_sb = ypool.tile([P, H], F32)
                nc.scalar.copy(y_sb[:, 0:512], py0[:])
                nc.vector.tensor_copy(y_sb[:, 512:1024], py1[:])
                nc.sync.dma_start(y_d[ms, :], y_sb[:])

    nc.compile()
    return nc


def _get_compiled():
    global _compiled
    if _compiled is None:
        _compiled = _build()
    return _compiled


def _numpy_fallback(hidden_states, gate_kernel, up_kernel, down_kernel, group_sizes):
    # Exact reference math on host; only used for unexpected group_sizes.
    out = np.empty((hidden_states.shape[0], down_kernel.shape[2]), np.float32)
    start = 0
    for e in range(gate_kernel.shape[0]):
        g = int(group_sizes[e])
        x = hidden_states[start:start + g]
        gate = x @ gate_kernel[e]
        up = x @ up_kernel[e]
        h = (gate / (1.0 + np.exp(-gate))) * up
        out[start:start + g] = h @ down_kernel[e]
        start += g
    out[start:] = 0.0
    return out


def _make_in_maps(hidden_states, gate_kernel, up_kernel, down_kernel):
    bf = ml_dtypes.bfloat16
    in_maps = []
    for e in range(E):
        x_e = hidden_states[e * T:(e + 1) * T]
        in_maps.append({
            "xT": np.ascontiguousarray(x_e.T).astype(bf),
            "wg": np.ascontiguousarray(gate_kernel[e]).astype(bf),
            "wu": np.ascontiguousarray(up_kernel[e]).astype(bf),
            "wd": np.ascontiguousarray(down_kernel[e]).astype(bf),
        })
    return in_maps


def profile_run(inputs, tmpdir=None):
    """Dev helper (not used by grading): run with NTFF tracing, return exec ns."""
    nc = _get_compiled()
    in_maps = _make_in_maps(
        np.asarray(inputs["hidden_states"], np.float32),
        np.asarray(inputs["gate_kernel"], np.float32),
        np.asarray(inputs["up_kernel"], np.float32),
        np.asarray(inputs["down_kernel"], np.float32),
    )
    res = run_bass_kernel_spmd(
        nc, in_maps, core_ids=list(range(E)), trace=True, tmpdir=tmpdir
    )
    return res.exec_time_ns


def kernel(hidden_states, gate_kernel, up_kernel, down_kernel, group_sizes):
    hidden_states = np.asarray(hidden_states, dtype=np.float32)
    gate_kernel = np.asarray(gate_kernel, dtype=np.float32)
    up_kernel = np.asarray(up_kernel, dtype=np.float32)
    down_kernel = np.asarray(down_kernel, dtype=np.float32)
    gs = np.asarray(group_sizes)

    if not (gs.shape == (E,) and np.all(gs == T)):
        return _numpy_fallback(
            hidden_states, gate_kernel, up_kernel, down_kernel, gs
        )

    nc = _get_compiled()
    in_maps = _make_in_maps(hidden_states, gate_kernel, up_kernel, down_kernel)
    res = run_bass_kernel_spmd(nc, in_maps, core_ids=list(range(E)))
    return np.concatenate([res.results[e]["y"] for e in range(E)], axis=0)
